# revision 7
# baseline (speedup 1.0000x reference)
"""Navier-Stokes PINO loss kernel for Trainium2 (8 NeuronCores, SPMD).

Contract: kernel(u_pred, u_prev) with full [4, 8, 2, 512, 512] fp32 inputs,
returns np.ndarray [3] = (physics_loss, pde_loss, div_loss).

Sharding: pure data-parallel over the 32 (B,T) pairs -> 4 pairs per core.
Each core computes per-partition partial sums of residual^2 and divergence^2;
the host reduces them in float64 and forms the three scalar losses.

Math (matching the jax reference exactly, periodic central differences, dx=1):
  res_x = (U - PU)/DT + U*u_x + V*u_y - NU*lap(U)
  res_y = (V - PV)/DT + U*v_x + V*v_y - NU*lap(V)
  div   = u_x + v_y
  pde_loss = mean(res_x^2) + mean(res_y^2);  div_loss = mean(div^2)
  physics_loss = pde_loss + LAMBDA_DIV*div_loss

On-chip formulation (per (b,t), channels batched where possible):
  layout: row r = 4*p + j  (p = partition 0..127, j = row slot 0..3),
  tile UV[p, c, jj, ww] with jj in 0..5 (y-halo slots 0 and 5) and
  ww in 0..513 (x-halo cols 0 and 513).
  gx = Xp - Xm, gy = Yp - Ym            (raw grads, = 2*grad)
  s1 = Xp + Xm, s2 = Yp + Ym
  Dn  = PU*(b/a) - U                    with a = 1/DT + 4*NU, b = 1/DT
  A1n = (U * (-0.5/a)) * gx
  A2n = (V * (-0.5/a)) * gy
  resn = Dn + (NU/a)*s1 + (NU/a)*s2 + A1n + A2n     ( = -res/a )
  acc_pde  += sum((a*resn)^2)           (ACT Square with scale=a)
  acc_div  += sum((0.5*(gx_u + gy_v))^2)
"""

import os
import sys

import numpy as np

for _p in ("/opt/trn_rl_repo",):
    if _p not in sys.path:
        sys.path.insert(0, _p)

from contextlib import ExitStack

import concourse.bass as bass
import concourse.tile as tile
from concourse import bacc, mybir
from concourse.bass_utils import run_bass_kernel_spmd

NCORES = 8
B, T, C, H, W = 4, 8, 2, 512, 512
BT = B * T                      # 32 (b,t) pairs
BT_PER_CORE = BT // NCORES      # 4
NU = 0.001
LAMBDA_DIV = 0.1
DT_ = 0.01
A_COEF = 1.0 / DT_ + 4.0 * NU   # 100.004
B_COEF = 1.0 / DT_              # 100.0

F32 = mybir.dt.float32
OP = mybir.AluOpType


def build_nc():
    nc = bacc.Bacc(
        "TRN2",
        target_bir_lowering=False,
        debug=False,
        enable_asserts=False,
        num_devices=NCORES,
    )
    up_d = nc.dram_tensor(
        "u_pred", [BT_PER_CORE, C, H, W], F32, kind="ExternalInput"
    ).ap()
    uv_d = nc.dram_tensor(
        "u_prev", [BT_PER_CORE, C, H, W], F32, kind="ExternalInput"
    ).ap()
    acc_d = nc.dram_tensor(
        "acc", [128, 2 * BT_PER_CORE], F32, kind="ExternalOutput"
    ).ap()

    with tile.TileContext(nc) as tc, ExitStack() as ctx:
        io = ctx.enter_context(tc.tile_pool(name="io", bufs=2))
        puv_pool = ctx.enter_context(tc.tile_pool(name="puvp", bufs=1))
        tp = ctx.enter_context(tc.tile_pool(name="tmp", bufs=1))
        accp = ctx.enter_context(tc.tile_pool(name="accp", bufs=1))

        accs = accp.tile([128, 2 * BT_PER_CORE], F32, name="accs")

        for bt in range(BT_PER_CORE):
            # ---- tiles --------------------------------------------------
            UV = io.tile([128, C, 6, 514], F32, tag="uv", name=f"uv{bt}")
            PUV = puv_pool.tile([128, C, 4, 512], F32, tag="puv", name=f"puv{bt}")
            gx = tp.tile([128, C, 4, 512], F32, tag="gx", name=f"gx{bt}")
            gy = tp.tile([128, C, 4, 512], F32, tag="gy", name=f"gy{bt}")
            tA = tp.tile([128, C, 4, 512], F32, tag="tA", name=f"tA{bt}")  # s1, q2
            tB = tp.tile([128, C, 4, 512], F32, tag="tB", name=f"tB{bt}")  # s2, q3
            tC = tp.tile([128, C, 4, 512], F32, tag="tC", name=f"tC{bt}")  # Dn, resn
            tD = tp.tile([128, C, 4, 512], F32, tag="tD", name=f"tD{bt}")  # q1, dv
            tE = tp.tile([128, C, 4, 512], F32, tag="tE", name=f"tE{bt}")  # A1n, A2n

            # ---- loads --------------------------------------------------
            for c in range(C):
                src = up_d[bt, c]  # [H, W]
                # body rows r = 4p + j  ->  UV[p, c, 1+j, 1:513]
                nc.sync.dma_start(
                    UV[:, c, 1:5, 1:513],
                    src.rearrange("(p j) w -> p j w", j=4),
                )
                # y-halo up (slot 0): row 4p-1; p>=1 <- rows 3,7,...,507
                nc.sync.dma_start(UV[1:128, c, 0, 1:513], src[3:508:4, :])
                # p=0 wraps to row 511
                nc.sync.dma_start(UV[0:1, c, 0, 1:513], src[511:512, :])
                # y-halo down (slot 5): row 4p+4; p<=126 <- rows 4,8,...,508
                nc.sync.dma_start(UV[0:127, c, 5, 1:513], src[4:509:4, :])
                # p=127 wraps to row 0
                nc.sync.dma_start(UV[127:128, c, 5, 1:513], src[0:1, :])
                # u_prev (no halo)
                nc.sync.dma_start(
                    PUV[:, c],
                    uv_d[bt, c].rearrange("(p j) w -> p j w", j=4),
                )

            # x-halo cols (body slots only): col 0 <- col 512, col 513 <- col 1
            for c in range(C):
                nc.scalar.copy(UV[:, c, 1:5, 0:1], UV[:, c, 1:5, 512:513])
                nc.scalar.copy(UV[:, c, 1:5, 513:514], UV[:, c, 1:5, 1:2])

            # ---- compute ------------------------------------------------
            # walrus caps compute-op APs at 3 dims, so every op with a
            # (padded, non-contiguous) UV view runs per-channel [128,4,512];
            # ops on packed temps flatten to 2D and run channel-batched.
            v = nc.vector
            g = nc.gpsimd
            Uv = UV[:, 0, 1:5, 1:513]
            Vv = UV[:, 1, 1:5, 1:513]
            for c in range(C):
                Cv = UV[:, c, 1:5, 1:513]
                Xp = UV[:, c, 1:5, 2:514]
                Xm = UV[:, c, 1:5, 0:512]
                Yp = UV[:, c, 2:6, 1:513]
                Ym = UV[:, c, 0:4, 1:513]
                v.tensor_sub(gx[:, c], Xp, Xm)
                v.tensor_sub(gy[:, c], Yp, Ym)
                g.tensor_add(tA[:, c], Xp, Xm)    # s1  (POOL)
                g.tensor_add(tB[:, c], Yp, Ym)    # s2  (POOL)
                # Dn = PU*(b/a) - U
                v.scalar_tensor_tensor(
                    tC[:, c], PUV[:, c], B_COEF / A_COEF, Cv,
                    op0=OP.mult, op1=OP.subtract,
                )
                # A1n_c = (U * (-0.5/a)) * gx_c
                v.scalar_tensor_tensor(
                    tE[:, c], Uv, -0.5 / A_COEF, gx[:, c],
                    op0=OP.mult, op1=OP.mult,
                )
            # q1 = (s1 * NU/a) + Dn
            v.scalar_tensor_tensor(
                tD[:], tA[:], NU / A_COEF, tC[:], op0=OP.mult, op1=OP.add
            )
            # q2 = (s2 * NU/a) + q1
            v.scalar_tensor_tensor(
                tA[:], tB[:], NU / A_COEF, tD[:], op0=OP.mult, op1=OP.add
            )
            # q3 = q2 + A1n  (POOL)
            g.tensor_add(tB[:], tA[:], tE[:])
            # A2n_c = (V * (-0.5/a)) * gy_c
            for c in range(C):
                v.scalar_tensor_tensor(
                    tE[:, c], Vv, -0.5 / A_COEF, gy[:, c],
                    op0=OP.mult, op1=OP.mult,
                )
            # resn = q3 + A2n
            v.tensor_add(tC[:], tB[:], tE[:])
            # acc_pde[bt] = sum((a*resn)^2) over free dims (both channels)
            nc.scalar.activation(
                tC[:],
                tC[:],
                mybir.ActivationFunctionType.Square,
                scale=float(A_COEF),
                accum_out=accs[:, bt : bt + 1],
            )
            # dv = gx_u + gy_v ; acc_div[bt] = sum((0.5*dv)^2)  (POOL)
            dvo = tD[:, 0, :, :]
            g.tensor_add(dvo, gx[:, 0, :, :], gy[:, 1, :, :])
            nc.scalar.activation(
                dvo,
                dvo,
                mybir.ActivationFunctionType.Square,
                scale=0.5,
                accum_out=accs[:, BT_PER_CORE + bt : BT_PER_CORE + bt + 1],
            )

        nc.sync.dma_start(acc_d, accs[:])

    nc.compile()
    return nc


_NC_CACHE = {}


def _get_nc():
    if "nc" not in _NC_CACHE:
        _NC_CACHE["nc"] = build_nc()
    return _NC_CACHE["nc"]


def kernel(u_pred: np.ndarray, u_prev: np.ndarray) -> np.ndarray:
    nc = _get_nc()
    up = np.ascontiguousarray(u_pred, dtype=np.float32).reshape(BT, C, H, W)
    uv = np.ascontiguousarray(u_prev, dtype=np.float32).reshape(BT, C, H, W)
    in_maps = []
    for k in range(NCORES):
        sl = slice(k * BT_PER_CORE, (k + 1) * BT_PER_CORE)
        in_maps.append(
            {
                "u_pred": np.ascontiguousarray(up[sl]),
                "u_prev": np.ascontiguousarray(uv[sl]),
            }
        )
    res = run_bass_kernel_spmd(
        nc,
        in_maps,
        core_ids=list(range(NCORES)),
        trace=bool(int(os.environ.get("NSPINO_TRACE", "0"))),
    )
    if res.exec_time_ns is not None:
        _NC_CACHE["exec_time_ns"] = res.exec_time_ns
    acc = np.stack([r["acc"] for r in res.results]).astype(np.float64)
    # acc[:, :, 0:4] -> per-(b,t) sum(res_x^2)+sum(res_y^2); [:, 4:8] -> sum(dv^2)
    n = float(BT * H * W)
    pde = acc[:, :, :BT_PER_CORE].sum() / n
    div = acc[:, :, BT_PER_CORE:].sum() / n
    phys = pde + LAMBDA_DIV * div
    return np.array([phys, pde, div], dtype=np.float32)


# revision 12
# speedup vs baseline: 1.6140x; 1.6140x over previous
"""Navier-Stokes PINO loss kernel for Trainium2 (8 NeuronCores, SPMD).

Contract: kernel(u_pred, u_prev) with full [4, 8, 2, 512, 512] fp32 inputs,
returns np.ndarray [3] = (physics_loss, pde_loss, div_loss).

Sharding: data-parallel over the 32 (B,T) pairs -> 4 per core. Each core
writes per-partition partial sums of residual^2 / divergence^2; the host
reduces in float64.

v2 design (per (b,t), row layout r = 4p + j):
  - u_pred loaded fp32 with x-halo cols (tile UV [128,2,4,514]).
  - bf16 working set via SWDGE cast-DMAs: UVb [128,2,6,512] (body + y-halo
    slots, partition-shifted casts), PUVb (u_prev, cast straight from DRAM).
  - DVE (bf16 2x where aligned): gx = Xp-Xm (fp32-in), gy, ys, A1 = U*gx,
    A2 = V*gy, D = Ub-PUb.
  - POOL: xs = Xp+Xm, div = gx_u + gy_v.
  - PE assembles the residual in PSUM with the constants folded into bf16
    diagonal weights:  res = 100*D - NU*xs - NU*ys + 0.5*A1 + 0.5*A2
    + 0.004*U   (= (U-PU)/DT + advection - NU*lap, since lap = xs+ys-4U).
  - ACT: Square+accumulate from PSUM (pde) and SBUF (div, scale 0.5).
Emulated-bf16 numpy check: loss rel err ~6e-6 vs fp32 reference.
"""

import os
import sys

import numpy as np

for _p in ("/opt/trn_rl_repo",):
    if _p not in sys.path:
        sys.path.insert(0, _p)

from contextlib import ExitStack

import concourse.bass as bass
import concourse.tile as tile
from concourse import bacc, mybir
from concourse.bass_utils import run_bass_kernel_spmd

NCORES = 8
B, T, C, H, W = 4, 8, 2, 512, 512
BT = B * T
BT_PER_CORE = BT // NCORES
NU = 0.001
LAMBDA_DIV = 0.1
DT_ = 0.01

F32 = mybir.dt.float32
BF16 = mybir.dt.bfloat16
OP = mybir.AluOpType

# PE diagonal weights (bf16): [100, -NU, 0.5, 4*NU]
_WVALS = [100.0, -NU, 0.5, 4.0 * NU]


def _weight_host() -> np.ndarray:
    import ml_dtypes

    w = np.zeros((4, 128, 128), dtype=np.float32)
    for k, val in enumerate(_WVALS):
        np.fill_diagonal(w[k], val)
    return np.ascontiguousarray(w.astype(ml_dtypes.bfloat16))


def build_nc():
    nc = bacc.Bacc(
        "TRN2",
        target_bir_lowering=False,
        debug=False,
        enable_asserts=False,
        num_devices=NCORES,
    )
    up_d = nc.dram_tensor(
        "u_pred", [BT_PER_CORE, C, H, W], F32, kind="ExternalInput"
    ).ap()
    uv_d = nc.dram_tensor(
        "u_prev", [BT_PER_CORE, C, H, W], F32, kind="ExternalInput"
    ).ap()
    w_d = nc.dram_tensor("wdiag", [4, 128, 128], BF16, kind="ExternalInput").ap()
    acc_d = nc.dram_tensor(
        "acc", [128, 3 * BT_PER_CORE], F32, kind="ExternalOutput"
    ).ap()

    with tile.TileContext(nc) as tc, ExitStack() as ctx:
        io = ctx.enter_context(tc.tile_pool(name="io", bufs=2))
        tp = ctx.enter_context(tc.tile_pool(name="tmp", bufs=2))
        onep = ctx.enter_context(tc.tile_pool(name="onep", bufs=1))
        psp = ctx.enter_context(tc.tile_pool(name="psp", bufs=1, space="PSUM"))

        accs = onep.tile([128, 3 * BT_PER_CORE], F32, name="accs")
        wt = onep.tile([128, 4, 128], BF16, name="wt")
        for k in range(4):
            nc.sync.dma_start(wt[:, k, :], w_d[k])
        W100, WNU, W05, W004 = (wt[:, k, :] for k in range(4))

        for bt in range(BT_PER_CORE):
            UV = io.tile([128, C, 4, 514], F32, tag="uv", name=f"uv{bt}")
            UVb = io.tile([128, C, 6, 512], BF16, tag="uvb", name=f"uvb{bt}")
            PUVb = io.tile([128, C, 4, 512], BF16, tag="puvb", name=f"puvb{bt}")
            gx = tp.tile([128, C, 4, 512], BF16, tag="gx", name=f"gx{bt}")
            gy = tp.tile([128, C, 4, 512], BF16, tag="gy", name=f"gy{bt}")
            xs = tp.tile([128, C, 4, 512], BF16, tag="xs", name=f"xs{bt}")
            ys = tp.tile([128, C, 4, 512], BF16, tag="ys", name=f"ys{bt}")
            A1 = tp.tile([128, C, 4, 512], BF16, tag="A1", name=f"A1{bt}")
            A2 = tp.tile([128, C, 4, 512], BF16, tag="A2", name=f"A2{bt}")
            Dt = tp.tile([128, C, 4, 512], BF16, tag="Dt", name=f"Dt{bt}")
            dv = tp.tile([128, 4, 512], BF16, tag="dv", name=f"dv{bt}", bufs=1)

            v, g, s = nc.vector, nc.gpsimd, nc.scalar

            for c in range(C):
                # fp32 body with x-halo cols
                nc.sync.dma_start(
                    UV[:, c, :, 1:513],
                    up_d[bt, c].rearrange("(p j) w -> p j w", j=4),
                )
                # u_prev straight to bf16 (SWDGE cast)
                g.dma_start(
                    PUVb[:, c],
                    uv_d[bt, c].rearrange("(p j) w -> p j w", j=4),
                )
            for c in range(C):
                # x-halo cols: col 0 <- col 512 (W 511), col 513 <- col 1 (W 0)
                s.copy(UV[:, c, :, 0:1], UV[:, c, :, 512:513])
                s.copy(UV[:, c, :, 513:514], UV[:, c, :, 1:2])
                # bf16 body cast (SBUF->SBUF SWDGE)
                g.dma_start(UVb[:, c, 1:5, :], UV[:, c, :, 1:513])
                # y-halos (partition-shifted casts):
                # slot 0 row 4p-1: p>=1 <- (p-1, j=3); p=0 <- (127, j=3)
                g.dma_start(UVb[1:128, c, 0, :], UV[0:127, c, 3, 1:513])
                g.dma_start(UVb[0:1, c, 0, :], UV[127:128, c, 3, 1:513])
                # slot 5 row 4p+4: p<=126 <- (p+1, j=0); p=127 <- (0, j=0)
                g.dma_start(UVb[0:127, c, 5, :], UV[1:128, c, 0, 1:513])
                g.dma_start(UVb[127:128, c, 5, :], UV[0:1, c, 0, 1:513])

            for c in range(C):
                Xp = UV[:, c, :, 2:514]
                Xm = UV[:, c, :, 0:512]
                Yp = UVb[:, c, 2:6, :]
                Ym = UVb[:, c, 0:4, :]
                Ub = UVb[:, 0, 1:5, :]
                Vb = UVb[:, 1, 1:5, :]
                v.tensor_sub(gx[:, c], Xp, Xm)          # fp32-in, bf16-out, 1x
                g.tensor_add(xs[:, c], Xp, Xm)          # POOL
                v.tensor_sub(gy[:, c], Yp, Ym)          # bf16 2x
                v.tensor_add(ys[:, c], Yp, Ym)          # bf16 2x
                v.tensor_mul(A1[:, c], Ub, gx[:, c])    # bf16 2x
                v.tensor_mul(A2[:, c], Vb, gy[:, c])    # bf16 2x
                v.tensor_sub(Dt[:, c], UVb[:, c, 1:5, :], PUVb[:, c])  # bf16 2x

            # PE: assemble residual in PSUM, weights carry the constants.
            psums = [
                psp.tile([128, 4, 512], F32, tag=f"ps{c}", name=f"ps{c}_{bt}")
                for c in range(C)
            ]
            groups = [
                (W100, Dt, False),
                (WNU, xs, False),
                (WNU, ys, False),
                (W05, A1, False),
                (W05, A2, False),
                (W004, None, True),  # 0.004 * U (body of UVb)
            ]
            n_g = len(groups)
            for gi, (wap, ten, is_u) in enumerate(groups):
                for c in range(C):
                    for j in range(4):
                        rhs = UVb[:, c, 1 + j, :] if is_u else ten[:, c, j, :]
                        nc.tensor.matmul(
                            psums[c][:, j, :],
                            wap,
                            rhs,
                            start=(gi == 0),
                            stop=(gi == n_g - 1),
                        )

            # pde: sum over both channels of res^2 (ACT Square + accum)
            for c in range(C):
                # out -> Dt (dead by now): avoids in-place read+write on the
                # same PSUM bank; the squared values themselves are unused.
                s.activation(
                    Dt[:, c],
                    psums[c][:],
                    mybir.ActivationFunctionType.Square,
                    accum_out=accs[:, 2 * bt + c : 2 * bt + c + 1],
                )
            # div = gx_u + gy_v (POOL), then sum (0.5*div)^2
            g.tensor_add(dv[:], gx[:, 0], gy[:, 1])
            s.activation(
                dv[:],
                dv[:],
                mybir.ActivationFunctionType.Square,
                scale=0.5,
                accum_out=accs[:, 2 * BT_PER_CORE + bt : 2 * BT_PER_CORE + bt + 1],
            )

        nc.sync.dma_start(acc_d, accs[:])

    nc.compile()
    return nc


_NC_CACHE = {}


def _get_nc():
    if "nc" not in _NC_CACHE:
        _NC_CACHE["nc"] = build_nc()
    return _NC_CACHE["nc"]


def kernel(u_pred: np.ndarray, u_prev: np.ndarray) -> np.ndarray:
    nc = _get_nc()
    up = np.ascontiguousarray(u_pred, dtype=np.float32).reshape(BT, C, H, W)
    uv = np.ascontiguousarray(u_prev, dtype=np.float32).reshape(BT, C, H, W)
    wh = _weight_host()
    in_maps = []
    for k in range(NCORES):
        sl = slice(k * BT_PER_CORE, (k + 1) * BT_PER_CORE)
        in_maps.append(
            {
                "u_pred": np.ascontiguousarray(up[sl]),
                "u_prev": np.ascontiguousarray(uv[sl]),
                "wdiag": wh,
            }
        )
    res = run_bass_kernel_spmd(
        nc,
        in_maps,
        core_ids=list(range(NCORES)),
        trace=bool(int(os.environ.get("NSPINO_TRACE", "0"))),
    )
    if res.exec_time_ns is not None:
        _NC_CACHE["exec_time_ns"] = res.exec_time_ns
    _NC_CACHE["last_results"] = res
    acc = np.stack([r["acc"] for r in res.results]).astype(np.float64)
    n = float(BT * H * W)
    pde = acc[:, :, : 2 * BT_PER_CORE].sum() / n
    div = acc[:, :, 2 * BT_PER_CORE :].sum() / n
    phys = pde + LAMBDA_DIV * div
    return np.array([phys, pde, div], dtype=np.float32)
